# revision 1
# baseline (speedup 1.0000x reference)
"""DualConsensusNet Trainium2 kernel: 3-layer RelCNN GNN on two graphs +
cosine match + Sinkhorn(10), node-sharded across 8 NeuronCores.

Self-contained: hardcodes all shapes from the problem spec.
"""
import numpy as np

import concourse.bass as bass
import concourse.bacc as bacc
import concourse.mybir as mybir
from concourse import tile
from concourse.bass_utils import run_bass_kernel_spmd

F32 = mybir.dt.float32
BF16 = mybir.dt.bfloat16
I16 = mybir.dt.int16

NCORES = 8
NS, NT = 4000, 4096
NP = 4096            # padded node count per graph
SH = 512             # nodes per core per graph
D_IN, D_H, N_LAYERS = 128, 256, 3
EPS = 1e-10
ALPHA = 20.0
SINK_ITERS = 10
WIN = 64             # node window width for segment-sum masks
NGRP = SH // WIN     # 8 windows per core per graph
CHUNK_BLK = 16       # gather chunk = 16 blocks = 2048 edges


def _prep_edges(edges):
    """Partition+sort edges for both aggregation directions.

    dir 0 (out1): target=dst, gather h[src].  dir 1 (out2): target=src,
    gather h[dst].
    """
    src, dst = edges[0].astype(np.int64), edges[1].astype(np.int64)
    out = []
    for d in range(2):
        tgt = dst if d == 0 else src
        gsrc = src if d == 0 else dst
        deg = np.bincount(tgt, minlength=NP).astype(np.float32)
        inv_deg = (1.0 / np.maximum(deg, 1.0)).astype(np.float32)
        per_core = []
        for k in range(NCORES):
            m = (tgt >= k * SH) & (tgt < (k + 1) * SH)
            t_loc = tgt[m] - k * SH
            g = gsrc[m]
            order = np.argsort(t_loc, kind="stable")
            per_core.append((t_loc[order], g[order]))
        B = np.zeros(NGRP, np.int64)
        runs = []
        for k in range(NCORES):
            t_loc, g = per_core[k]
            cnt = np.bincount(t_loc // WIN, minlength=NGRP)
            runs.append(cnt)
            B = np.maximum(B, (cnt + 127) // 128)
        B = np.maximum(B, 1)
        nblk = int(B.sum())
        pad_blk = (-nblk) % CHUNK_BLK
        B[-1] += pad_blk
        nblk += pad_blk
        epad = nblk * 128
        idx_all, rel_all = [], []
        for k in range(NCORES):
            t_loc, g = per_core[k]
            idx = np.zeros(epad, np.int64)
            rel = np.full(epad, -1.0, np.float32)  # idx 0 (real row), rel -1 => mask 0
            pos = 0
            start = 0
            for gi in range(NGRP):
                cnt = int(runs[k][gi])
                idx[pos:pos + cnt] = g[start:start + cnt]
                rel[pos:pos + cnt] = (t_loc[start:start + cnt] % WIN).astype(np.float32)
                start += cnt
                pos += int(B[gi]) * 128
            idx_all.append(idx)
            rel_all.append(rel)
        blk_win = np.repeat(np.arange(NGRP), B)
        out.append(dict(B=B, nblk=nblk, blk_win=blk_win,
                        idx=idx_all, rel=rel_all, inv_deg=inv_deg))
    return out


def _wrap_idx(idx):
    e = idx.shape[0]
    im = np.zeros((128, e // 16), np.int16)
    w = idx.reshape(e // 16, 16).T.astype(np.int16)
    for g in range(8):
        im[g * 16:(g + 1) * 16, :] = w
    return im


def _rel_tile(rel):
    e = rel.shape[0]
    return np.ascontiguousarray(rel.reshape(e // 128, 128).T)


def build_program(meta_s, meta_t):
    nc = bacc.Bacc(None, target_bir_lowering=False, debug=False,
                   num_devices=NCORES, num_swdge_queues=4)
    metas = {"s": meta_s, "t": meta_t}
    FAN = [D_IN, D_H, D_H]

    xs_sh = nc.dram_tensor("xs_sh", [SH, D_IN], F32, kind="ExternalInput")
    xt_sh = nc.dram_tensor("xt_sh", [SH, D_IN], F32, kind="ExternalInput")
    Ws = {}
    for l in range(N_LAYERS):
        f = FAN[l]
        for nm in ("W1", "W2", "Wr"):
            Ws[f"{nm}_{l}"] = nc.dram_tensor(f"{nm}_{l}", [f, D_H], F32,
                                             kind="ExternalInput")
        Ws[f"br_{l}"] = nc.dram_tensor(f"br_{l}", [D_H, 1], F32,
                                       kind="ExternalInput")
    final_w = nc.dram_tensor("final_w", [D_IN + 3 * D_H, D_H], F32,
                             kind="ExternalInput")
    final_b = nc.dram_tensor("final_b", [D_H, 1], F32, kind="ExternalInput")

    ein = {}
    for g in ("s", "t"):
        for d in range(2):
            m = metas[g][d]
            e = m["nblk"] * 128
            ein[f"idx_{g}{d}"] = nc.dram_tensor(f"idx_{g}{d}", [128, e // 16],
                                                I16, kind="ExternalInput")
            ein[f"rel_{g}{d}"] = nc.dram_tensor(f"rel_{g}{d}", [128, m["nblk"]],
                                                F32, kind="ExternalInput")
            ein[f"dg_{g}{d}"] = nc.dram_tensor(f"dg_{g}{d}", [128, SH // 128],
                                               F32, kind="ExternalInput")
    iota_in = nc.dram_tensor("iota_in", [128, WIN], F32, kind="ExternalInput")
    ident_in = nc.dram_tensor("ident_in", [128, 128], F32, kind="ExternalInput")
    valid_s = nc.dram_tensor("valid_s", [128, SH // 128], F32,
                             kind="ExternalInput")
    out_rows = nc.dram_tensor("out_rows", [SH, NP], F32, kind="ExternalOutput")

    tabs = {}
    for l in range(N_LAYERS):
        w = 2 * FAN[l]
        for g in ("s", "t"):
            tin = nc.dram_tensor(f"tab_in_{l}{g}", [SH, w], BF16)
            tout = nc.dram_tensor(f"tab_out_{l}{g}", [NP, w], BF16,
                                  addr_space="Shared")
            tabs[(l, g)] = (tin, tout, w)
    hfin_in = nc.dram_tensor("hfin_in", [D_H, SH], F32)
    hfin_out = nc.dram_tensor("hfin_out", [NCORES * D_H, SH], F32,
                              addr_space="Shared")
    cs_in = [nc.dram_tensor(f"cs_in_{i}", [1, NP], F32) for i in range(5)]
    scr_nrm = nc.dram_tensor("scr_nrm", [1, SH], F32)
    scr_inv = nc.dram_tensor("scr_inv", [1, SH], F32)
    scr_r = nc.dram_tensor("scr_r", [1, SH], F32)
    scr_c = nc.dram_tensor("scr_c", [1, NP], F32)
    cs_out = [nc.dram_tensor(f"cs_out_{i}", [1, NP], F32,
                             addr_space="Shared") for i in range(5)]

    RG = [list(range(NCORES))]

    with tile.TileContext(nc) as tc:
        with (
            tc.tile_pool(name="persist", bufs=1) as P,
            tc.tile_pool(name="mid", bufs=1) as MID,
            tc.tile_pool(name="psB", bufs=1, space="PSUM") as PSB,
            tc.tile_pool(name="psC", bufs=2, space="PSUM") as PSC,
        ):
            iota = P.tile([128, WIN], F32)
            nc.sync.dma_start(iota[:], iota_in[:])
            ident = P.tile([128, 128], F32)
            nc.sync.dma_start(ident[:], ident_in[:])
            ebias = P.tile([128, 1], F32)
            nc.vector.memset(ebias[:], ALPHA * EPS)
            ones = P.tile([128, 1], F32)
            nc.vector.memset(ones[:], 1.0)
            ones1 = P.tile([1, 128], F32)
            nc.vector.memset(ones1[:], 1.0)
            vs = P.tile([128, SH // 128], F32)
            nc.sync.dma_start(vs[:], valid_s[:])
            hfinT = {}

            # ============ GNN phase (pool closes before sinkhorn) ========
            with (
                tc.tile_pool(name="gnn", bufs=1) as G,
                tc.tile_pool(name="work", bufs=1) as W,
                tc.tile_pool(name="vals", bufs=3) as V,
                tc.tile_pool(name="psA", bufs=1, space="PSUM") as PSA,
            ):
                wt = {}
                for l in range(N_LAYERS):
                    f = FAN[l]
                    for nm in ("W1", "W2", "Wr"):
                        t = G.tile([128, f // 128, D_H], F32, tag=f"{nm}_{l}")
                        for kt in range(f // 128):
                            nc.sync.dma_start(
                                t[:, kt, :],
                                Ws[f"{nm}_{l}"][kt * 128:(kt + 1) * 128, :])
                        wt[f"{nm}_{l}"] = t
                    b = G.tile([128, 2], F32, tag=f"br_{l}")
                    for h in range(2):
                        nc.sync.dma_start(b[:, h:h + 1],
                                          Ws[f"br_{l}"][h * 128:(h + 1) * 128, :])
                    wt[f"br_{l}"] = b
                fw = G.tile([128, 7, D_H], F32)
                for kt in range(7):
                    nc.sync.dma_start(fw[:, kt, :],
                                      final_w[kt * 128:(kt + 1) * 128, :])
                fb = G.tile([128, 2], F32)
                for h in range(2):
                    nc.sync.dma_start(fb[:, h:h + 1],
                                      final_b[h * 128:(h + 1) * 128, :])

                em = {}
                masks = {}
                for g in ("s", "t"):
                    for d in range(2):
                        m = metas[g][d]
                        e = m["nblk"] * 128
                        it = G.tile([128, e // 16], I16, tag=f"idx_{g}{d}")
                        nc.sync.dma_start(it[:], ein[f"idx_{g}{d}"][:])
                        rl = G.tile([128, m["nblk"]], F32, tag=f"rel_{g}{d}")
                        nc.sync.dma_start(rl[:], ein[f"rel_{g}{d}"][:])
                        dg = G.tile([128, SH // 128], F32, tag=f"dg_{g}{d}")
                        nc.sync.dma_start(dg[:], ein[f"dg_{g}{d}"][:])
                        em[(g, d)] = (it, rl, dg, m)
                        mk = G.tile([128, m["nblk"], WIN], BF16,
                                    tag=f"mask_{g}{d}")
                        for b in range(m["nblk"]):
                            nc.vector.tensor_scalar(
                                mk[:, b, :], iota[:], rl[:, b:b + 1], None,
                                mybir.AluOpType.is_equal)
                        masks[(g, d)] = mk

                hT = {}
                for g, xin in (("s", xs_sh), ("t", xt_sh)):
                    t = G.tile([128, 1, SH], F32, tag=f"hT0_{g}")
                    nc.sync.dma_start(t[:, 0, :], xin.ap().rearrange("n f -> f n"))
                    hT[g] = t
                hist = {"s": [], "t": []}

                def write_table(l, g):
                    tin, tout, wdt = tabs[(l, g)]
                    f = FAN[l]
                    nm_t = W.tile([128, SH // 128, f], F32, tag="tab_nm")
                    for kt in range(f // 128):
                        for ntile in range(SH // 128):
                            pst = PSC.tile([128, 128], F32, tag="tr")
                            nc.tensor.transpose(
                                pst[:],
                                hT[g][:, kt, ntile * 128:(ntile + 1) * 128],
                                ident[:])
                            nc.scalar.copy(
                                nm_t[:, ntile, kt * 128:(kt + 1) * 128],
                                pst[:])
                    hi = W.tile([128, SH // 128, f], BF16, tag="tab_hi")
                    lo_f = W.tile([128, SH // 128, f], F32, tag="tab_lof")
                    lo = W.tile([128, SH // 128, f], BF16, tag="tab_lo")
                    nc.vector.tensor_copy(hi[:], nm_t[:])
                    nc.vector.tensor_tensor(lo_f[:], nm_t[:], hi[:],
                                            mybir.AluOpType.subtract)
                    nc.vector.tensor_copy(lo[:], lo_f[:])
                    for ntile in range(SH // 128):
                        nc.sync.dma_start(
                            tin[ntile * 128:(ntile + 1) * 128, 0:f],
                            hi[:, ntile, :])
                        nc.sync.dma_start(
                            tin[ntile * 128:(ntile + 1) * 128, f:2 * f],
                            lo[:, ntile, :])
                    nc.gpsimd.collective_compute(
                        "AllGather", mybir.AluOpType.bypass,
                        replica_groups=RG,
                        ins=[tin.ap().opt()], outs=[tout.ap().opt()])

                def aggregate(g, d, l):
                    tin, tout, wdt = tabs[(l, g)]
                    f = FAN[l]
                    it, rl, dg, m = em[(g, d)]
                    mk = masks[(g, d)]
                    nblk = m["nblk"]
                    blk_win = m["blk_win"]
                    pst = [PSA.tile([128, f], F32, tag=f"agg{q}",
                                    name=f"aggps_{g}{d}{l}_{q}")
                           for q in range(4)]
                    started = [False] * NGRP
                    for c in range(nblk // CHUNK_BLK):
                        vhi = V.tile([128, CHUNK_BLK, f], BF16, tag="vhi")
                        vlo = V.tile([128, CHUNK_BLK, f], BF16, tag="vlo")
                        i0 = c * CHUNK_BLK * 128 // 16
                        i1 = (c + 1) * CHUNK_BLK * 128 // 16
                        for vi, (vt, foff) in enumerate(((vhi, 0),
                                                         (vlo, f))):
                            nc.gpsimd.dma_gather(
                                vt[:], tout[:, foff:foff + f], it[:, i0:i1],
                                CHUNK_BLK * 128, CHUNK_BLK * 128, f,
                                elem_step=wdt, single_packet=False,
                                queue_num=(2 * c + vi) % 4)
                        for bb in range(CHUNK_BLK):
                            b = c * CHUNK_BLK + bb
                            w = int(blk_win[b])
                            q, half = w // 2, w % 2
                            st = not started[w]
                            started[w] = True
                            last = (b == nblk - 1 or blk_win[b + 1] != w)
                            nc.tensor.matmul(
                                pst[q][half * 64:(half + 1) * 64, :],
                                mk[:, b, :], vhi[:, bb, :], start=st,
                                stop=False)
                            nc.tensor.matmul(
                                pst[q][half * 64:(half + 1) * 64, :],
                                mk[:, b, :], vlo[:, bb, :], start=False,
                                stop=last)
                    agg = W.tile([128, SH // 128, f], F32, tag=f"agg_nm{d}")
                    for q in range(SH // 128):
                        nc.vector.tensor_scalar_mul(agg[:, q, :], pst[q][:],
                                                    dg[:, q:q + 1])
                    return agg

                def to_featmajor(agg, f, tag):
                    at = W.tile([128, f // 128, SH], F32, tag=tag)
                    for kt in range(f // 128):
                        for ntile in range(SH // 128):
                            pst = PSC.tile([128, 128], F32, tag="tr")
                            nc.tensor.transpose(
                                pst[:], agg[:, ntile, kt * 128:(kt + 1) * 128],
                                ident[:])
                            nc.scalar.copy(
                                at[:, kt, ntile * 128:(ntile + 1) * 128],
                                pst[:])
                    return at

                write_table(0, "s")
                write_table(0, "t")
                for l in range(N_LAYERS):
                    f = FAN[l]
                    for g in ("s", "t"):
                        hist[g].append(hT[g])
                        a1 = aggregate(g, 0, l)
                        a2 = aggregate(g, 1, l)
                        a1t = to_featmajor(a1, f, "a1t")
                        a2t = to_featmajor(a2, f, "a2t")
                        hn = G.tile([128, 2, SH], F32, tag=f"hT{l + 1}_{g}")
                        for mt in range(2):
                            pp = PSB.tile([128, SH], F32, tag="pre")
                            for kt in range(f // 128):
                                nc.tensor.matmul(
                                    pp[:],
                                    wt[f"Wr_{l}"][:, kt, mt * 128:(mt + 1) * 128],
                                    hT[g][:, kt, :], start=(kt == 0),
                                    stop=False)
                            for kt in range(f // 128):
                                nc.tensor.matmul(
                                    pp[:],
                                    wt[f"W1_{l}"][:, kt, mt * 128:(mt + 1) * 128],
                                    a1t[:, kt, :], start=False, stop=False)
                            for kt in range(f // 128):
                                nc.tensor.matmul(
                                    pp[:],
                                    wt[f"W2_{l}"][:, kt, mt * 128:(mt + 1) * 128],
                                    a2t[:, kt, :], start=False,
                                    stop=(kt == f // 128 - 1))
                            nc.scalar.activation(
                                hn[:, mt, :], pp[:],
                                mybir.ActivationFunctionType.Relu,
                                bias=wt[f"br_{l}"][:, mt:mt + 1], scale=1.0)
                        hT[g] = hn
                        if l + 1 < N_LAYERS:
                            write_table(l + 1, g)

                # final linear + l2norm; t first so its AllGather overlaps
                # the s-side final compute
                for g in ("t", "s"):
                    hist[g].append(hT[g])
                    rhs = []
                    for t in hist[g]:
                        for kt in range(t[:].shape[1]):
                            rhs.append(t[:, kt, :])
                    hf = MID.tile([128, 2, SH], F32, tag=f"hfin_{g}")
                    for mt in range(2):
                        pp = PSB.tile([128, SH], F32, tag="pre")
                        for kt in range(7):
                            nc.tensor.matmul(
                                pp[:], fw[:, kt, mt * 128:(mt + 1) * 128],
                                rhs[kt], start=(kt == 0), stop=(kt == 6))
                        nc.scalar.copy(hf[:, mt, :], pp[:])
                        nc.vector.tensor_scalar_add(hf[:, mt, :], hf[:, mt, :],
                                                    fb[:, mt:mt + 1])
                    sq = W.tile([128, 2, SH], F32, tag="sq")
                    nc.scalar.activation(sq[:, 0, :], hf[:, 0, :],
                                         mybir.ActivationFunctionType.Square)
                    nc.scalar.activation(sq[:, 1, :], hf[:, 1, :],
                                         mybir.ActivationFunctionType.Square)
                    nrm = PSA.tile([1, SH], F32, tag="nrm")
                    nc.tensor.matmul(nrm[:], ones[:], sq[:, 0, :], start=True,
                                     stop=False)
                    nc.tensor.matmul(nrm[:], ones[:], sq[:, 1, :], start=False,
                                     stop=True)
                    nrs = W.tile([1, SH], F32, tag="nrs")
                    nc.scalar.activation(nrs[:], nrm[:],
                                         mybir.ActivationFunctionType.Sqrt)
                    nr2 = W.tile([128, SH // 128], F32, tag="nr2")
                    nc.sync.dma_start(scr_nrm.ap(), nrs[:])
                    nc.sync.dma_start(
                        nr2[:], scr_nrm[0, :].rearrange("(c b) -> b c", b=128))
                    nc.vector.tensor_scalar_max(nr2[:], nr2[:], 1e-12)
                    inv = W.tile([128, SH // 128], F32, tag="inv")
                    nc.vector.reciprocal(inv[:], nr2[:])
                    if g == "s":
                        nc.vector.tensor_tensor(inv[:], inv[:], vs[:],
                                                mybir.AluOpType.mult)
                    invr = W.tile([1, SH], F32, tag="invr")
                    nc.sync.dma_start(
                        scr_inv[0, :].rearrange("(c b) -> b c", b=128), inv[:])
                    nc.sync.dma_start(invr[:], scr_inv.ap())
                    invb = W.tile([128, SH], F32, tag="invb")
                    bcp = PSA.tile([128, SH], F32, tag="nrm", name="bcp")
                    nc.tensor.matmul(bcp[:], ones1[:], invr[:], start=True,
                                     stop=True)
                    nc.vector.tensor_copy(invb[:], bcp[:])
                    for mt in range(2):
                        nc.vector.tensor_tensor(hf[:, mt, :], hf[:, mt, :],
                                                invb[:], mybir.AluOpType.mult)
                    hfinT[g] = hf
                    if g == "t":
                        for mt in range(2):
                            nc.sync.dma_start(
                                hfin_in[mt * 128:(mt + 1) * 128, :],
                                hf[:, mt, :])
                        nc.gpsimd.collective_compute(
                            "AllGather", mybir.AluOpType.bypass,
                            replica_groups=RG,
                            ins=[hfin_in.ap().opt()],
                            outs=[hfin_out.ap().opt()])

            # ============ match + sinkhorn phase ============

            with (
                tc.tile_pool(name="sink", bufs=1) as S,
                tc.tile_pool(name="work2", bufs=1) as W2,
                tc.tile_pool(name="psS", bufs=1, space="PSUM") as PSS,
            ):
                m2_pool = tc.tile_pool(name="m2", bufs=1)
                M2 = m2_pool.__enter__()
                htn = M2.tile([128, 2, NP], F32)
                for r in range(NCORES):
                    nc.sync.dma_start(
                        htn[:, :, r * SH:(r + 1) * SH],
                        hfin_out[r * D_H:(r + 1) * D_H, :].rearrange(
                            "(h p) c -> p h c", p=128))
                S0 = [S.tile([128, NP], F32, tag=f"S0_{q}", name=f"S0_{q}") for q in range(4)]
                T0 = [S.tile([128, SH], F32, tag=f"T0_{q}", name=f"T0_{q}") for q in range(32)]
                for q in range(4):
                    for nchk in range(NP // 512):
                        pp = PSB.tile([128, SH], F32, tag="pre")
                        for kt in range(2):
                            nc.tensor.matmul(
                                pp[:], hfinT["s"][:, kt, q * 128:(q + 1) * 128],
                                htn[:, kt, nchk * 512:(nchk + 1) * 512],
                                start=(kt == 0), stop=(kt == 1))
                        nc.scalar.activation(
                            S0[q][:, nchk * 512:(nchk + 1) * 512], pp[:],
                            mybir.ActivationFunctionType.Exp,
                            bias=ebias[:, 0:1], scale=ALPHA)
                for q in range(4):
                    for jt in range(32):
                        pst = PSC.tile([128, 128], F32, tag="tr")
                        nc.tensor.transpose(
                            pst[:], S0[q][:, jt * 128:(jt + 1) * 128], ident[:])
                        if jt % 2 == 0:
                            nc.scalar.copy(T0[jt][:, q * 128:(q + 1) * 128],
                                           pst[:])
                        else:
                            nc.vector.tensor_copy(
                                T0[jt][:, q * 128:(q + 1) * 128], pst[:])

                m2_pool.__exit__(None, None, None)
                rt = S.tile([128, 4], F32)
                ct = S.tile([128, 32], F32)
                nc.vector.memset(rt[:], 1.0)
                csum_i = 0
                for it_i in range(SINK_ITERS):
                    if it_i % 2 == 0:
                        part = W2.tile([1, NP], F32, tag="part")
                        for nchk in range(NP // 512):
                            pp = PSS.tile([1, 512], F32, tag="cs")
                            for q in range(4):
                                nc.tensor.matmul(
                                    pp[:], rt[:, q:q + 1],
                                    S0[q][:, nchk * 512:(nchk + 1) * 512],
                                    start=(q == 0), stop=(q == 3))
                            nc.scalar.copy(
                                part[:, nchk * 512:(nchk + 1) * 512], pp[:])
                        nc.sync.dma_start(cs_in[csum_i][:], part[:])
                        nc.gpsimd.collective_compute(
                            "AllReduce", mybir.AluOpType.add,
                            replica_groups=RG,
                            ins=[cs_in[csum_i].ap().opt()],
                            outs=[cs_out[csum_i].ap().opt()])
                        ssum = W2.tile([128, 32], F32, tag="ssum")
                        nc.sync.dma_start(
                            ssum[:],
                            cs_out[csum_i][0, :].rearrange("(f p) -> p f",
                                                           p=128))
                        nc.vector.reciprocal(ct[:], ssum[:])
                        csum_i += 1
                    else:
                        pp = PSS.tile([1, SH], F32, tag="rs")
                        for jt in range(32):
                            nc.tensor.matmul(pp[:], ct[:, jt:jt + 1], T0[jt][:],
                                             start=(jt == 0), stop=(jt == 31))
                        rr = W2.tile([1, SH], F32, tag="rr")
                        nc.scalar.copy(rr[:], pp[:])
                        r2 = W2.tile([128, 4], F32, tag="r2")
                        nc.sync.dma_start(scr_r.ap(), rr[:])
                        nc.sync.dma_start(
                            r2[:], scr_r[0, :].rearrange("(c b) -> b c", b=128))
                        nc.vector.reciprocal(rt[:], r2[:])

                fin_cm = tc.tile_pool(name="fin", bufs=1)
                FIN = fin_cm.__enter__()
                crow = FIN.tile([1, NP], F32, tag="crow")
                nc.sync.dma_start(
                    scr_c[0, :].rearrange("(c b) -> b c", b=128), ct[:])
                nc.sync.dma_start(crow[:], scr_c.ap())
                cb = FIN.tile([128, NP], F32, tag="cb")
                for ch in range(NP // 512):
                    cbp = PSS.tile([128, 512], F32, tag="cb", name="cbp")
                    nc.tensor.matmul(cbp[:], ones1[:],
                                     crow[:, ch * 512:(ch + 1) * 512],
                                     start=True, stop=True)
                    nc.vector.tensor_copy(cb[:, ch * 512:(ch + 1) * 512],
                                          cbp[:])
                for q in range(4):
                    outt = FIN.tile([128, NP], F32, tag="outt")
                    nc.vector.scalar_tensor_tensor(
                        outt[:], S0[q][:], rt[:, q:q + 1], cb[:],
                        mybir.AluOpType.mult, mybir.AluOpType.mult)
                    nc.sync.dma_start(out_rows[q * 128:(q + 1) * 128, :],
                                      outt[:])
                fin_cm.__exit__(None, None, None)

    nc.compile()
    return nc


def kernel(**inputs):
    x_s = np.asarray(inputs["x_s"], np.float32)
    x_t = np.asarray(inputs["x_t"], np.float32)
    meta_s = _prep_edges(np.asarray(inputs["edges"]))
    meta_t = _prep_edges(np.asarray(inputs["edget"]))
    nc = build_program(meta_s, meta_t)

    xs_pad = np.zeros((NP, D_IN), np.float32)
    xs_pad[:NS] = x_s
    xt_pad = x_t

    iota_np = np.ascontiguousarray(
        np.broadcast_to(np.arange(WIN, dtype=np.float32), (128, WIN)))
    ident_np = np.eye(128, dtype=np.float32)
    in_maps = []
    for k in range(NCORES):
        m = dict(
            xs_sh=np.ascontiguousarray(xs_pad[k * SH:(k + 1) * SH]),
            xt_sh=np.ascontiguousarray(xt_pad[k * SH:(k + 1) * SH]),
            final_w=np.asarray(inputs["final_w"], np.float32),
            final_b=np.asarray(inputs["final_b"], np.float32).reshape(D_H, 1),
            iota_in=iota_np,
            ident_in=ident_np,
        )
        for l in range(N_LAYERS):
            for nm in ("W1", "W2", "Wr"):
                m[f"{nm}_{l}"] = np.asarray(inputs[f"{nm}_{l}"], np.float32)
            m[f"br_{l}"] = np.asarray(inputs[f"br_{l}"],
                                      np.float32).reshape(D_H, 1)
        for g, meta in (("s", meta_s), ("t", meta_t)):
            for d in range(2):
                md = meta[d]
                m[f"idx_{g}{d}"] = _wrap_idx(md["idx"][k])
                m[f"rel_{g}{d}"] = _rel_tile(md["rel"][k])
                dgk = md["inv_deg"][k * SH:(k + 1) * SH]
                m[f"dg_{g}{d}"] = np.ascontiguousarray(
                    dgk.reshape(SH // 128, 128).T)
        vld = np.zeros(SH, np.float32)
        n_real = max(0, min(SH, NS - k * SH))
        vld[:n_real] = 1.0
        m["valid_s"] = np.ascontiguousarray(vld.reshape(SH // 128, 128).T)
        in_maps.append(m)

    res = run_bass_kernel_spmd(nc, in_maps, list(range(NCORES)))
    rows = np.concatenate([res.results[k]["out_rows"] for k in range(NCORES)],
                          axis=0)
    kernel._last = (nc, in_maps)
    return rows[:NS].astype(np.float32)



# revision 4
# speedup vs baseline: 2.6393x; 2.6393x over previous
"""DualConsensusNet Trainium2 kernel: 3-layer RelCNN GNN on two graphs +
cosine match + Sinkhorn(10), node-sharded across 8 NeuronCores.

Self-contained: hardcodes all shapes from the problem spec.

Wire-transfer optimized: the axon tunnel dominates wall time, so inputs
are packed/compressed (weights sharded 8-way + on-device AllGather,
gather indices shipped compact and replicated on device, rel tables as
uint8, iota/identity generated on device) and the output matrix ships
as bf16.
"""
import numpy as np

import concourse.bass as bass
import concourse.bacc as bacc
import concourse.mybir as mybir
from concourse import tile
from concourse.bass_utils import run_bass_kernel_spmd

F32 = mybir.dt.float32
BF16 = mybir.dt.bfloat16
I16 = mybir.dt.int16
I32 = mybir.dt.int32
U8 = mybir.dt.uint8

NCORES = 8
NS, NT = 4000, 4096
NP = 4096            # padded node count per graph
SH = 512             # nodes per core per graph
D_IN, D_H, N_LAYERS = 128, 256, 3
EPS = 1e-10
ALPHA = 20.0
SINK_ITERS = 10
WIN = 64             # node window width for segment-sum masks
NGRP = SH // WIN     # 8 windows per core per graph
CHUNK_BLK = 16       # gather chunk = 16 blocks = 2048 edges
FAN = [D_IN, D_H, D_H]

# packed-weight row offsets (rows of 256 f32)
WOFF = {}
_off = 0
for _l in range(N_LAYERS):
    for _nm in ("W1", "W2", "Wr"):
        WOFF[f"{_nm}_{_l}"] = _off
        _off += FAN[_l]
WOFF["final_w"] = _off
_off += D_IN + 3 * D_H
WROWS = _off                      # 2816
WSH = (WROWS + 8 + NCORES - 1) // NCORES  # 353 rows/core (pads to 2824)
WPAD = WSH * NCORES

# misc pack columns: 0-5 br_l (col 2l+h), 6-7 final_b, 8-11 valid_s,
# 12-27 inv_deg for (s,0),(s,1),(t,0),(t,1)
MISC_COLS = 28
DG_BASE = {("s", 0): 12, ("s", 1): 16, ("t", 0): 20, ("t", 1): 24}


def _prep_edges(edges):
    """Partition+sort edges for both aggregation directions.

    dir 0 (out1): target=dst, gather h[src].  dir 1 (out2): target=src,
    gather h[dst].
    """
    src, dst = edges[0].astype(np.int64), edges[1].astype(np.int64)
    out = []
    for d in range(2):
        tgt = dst if d == 0 else src
        gsrc = src if d == 0 else dst
        deg = np.bincount(tgt, minlength=NP).astype(np.float32)
        inv_deg = (1.0 / np.maximum(deg, 1.0)).astype(np.float32)
        per_core = []
        for k in range(NCORES):
            m = (tgt >= k * SH) & (tgt < (k + 1) * SH)
            t_loc = tgt[m] - k * SH
            g = gsrc[m]
            order = np.argsort(t_loc, kind="stable")
            per_core.append((t_loc[order], g[order]))
        B = np.zeros(NGRP, np.int64)
        runs = []
        for k in range(NCORES):
            t_loc, g = per_core[k]
            cnt = np.bincount(t_loc // WIN, minlength=NGRP)
            runs.append(cnt)
            B = np.maximum(B, (cnt + 127) // 128)
        B = np.maximum(B, 1)
        nblk = int(B.sum())
        pad_blk = (-nblk) % CHUNK_BLK
        B[-1] += pad_blk
        nblk += pad_blk
        epad = nblk * 128
        idx_all, rel_all = [], []
        for k in range(NCORES):
            t_loc, g = per_core[k]
            idx = np.zeros(epad, np.int64)
            rel = np.full(epad, 255, np.int64)  # idx 0 (real row), rel 255 => mask 0
            pos = 0
            start = 0
            for gi in range(NGRP):
                cnt = int(runs[k][gi])
                idx[pos:pos + cnt] = g[start:start + cnt]
                rel[pos:pos + cnt] = t_loc[start:start + cnt] % WIN
                start += cnt
                pos += int(B[gi]) * 128
            idx_all.append(idx)
            rel_all.append(rel)
        blk_win = np.repeat(np.arange(NGRP), B)
        out.append(dict(B=B, nblk=nblk, blk_win=blk_win,
                        idx=idx_all, rel=rel_all, inv_deg=inv_deg))
    return out


def _wrap_idx(idx):
    e = idx.shape[0]
    return np.ascontiguousarray(idx.reshape(e // 16, 16).T.astype(np.int16))


def _rel_tile(rel):
    e = rel.shape[0]
    return np.ascontiguousarray(rel.reshape(e // 128, 128).T.astype(np.uint8))


def build_program(meta_s, meta_t):
    nc = bacc.Bacc(None, target_bir_lowering=False, debug=False,
                   num_devices=NCORES, num_swdge_queues=4)
    metas = {"s": meta_s, "t": meta_t}

    # --- packed external inputs (wire bytes are the bottleneck) ---
    xin = nc.dram_tensor("xin", [2 * SH, D_IN], F32, kind="ExternalInput")
    wpk_in = nc.dram_tensor("wpk_in", [WSH, D_H], F32, kind="ExternalInput")
    misc_in = nc.dram_tensor("misc_in", [128, MISC_COLS], F32,
                             kind="ExternalInput")
    idx_cols = sum(metas[g][d]["nblk"] * 8 for g in ("s", "t")
                   for d in range(2))
    rel_cols = sum(metas[g][d]["nblk"] for g in ("s", "t") for d in range(2))
    idxp = nc.dram_tensor("idxp", [16, idx_cols], I16, kind="ExternalInput")
    relp = nc.dram_tensor("relp", [128, rel_cols], U8, kind="ExternalInput")

    out_rows = nc.dram_tensor("out_rows", [SH, NP], BF16,
                              kind="ExternalOutput")

    wpk_b = nc.dram_tensor("wpk_b", [WSH, D_H], F32)
    wpk_out = nc.dram_tensor("wpk_out", [WPAD, D_H], F32, addr_space="Shared")

    tabs = {}
    for l in range(N_LAYERS):
        w = 2 * FAN[l]
        for g in ("s", "t"):
            tin = nc.dram_tensor(f"tab_in_{l}{g}", [SH, w], BF16)
            tout = nc.dram_tensor(f"tab_out_{l}{g}", [NP, w], BF16,
                                  addr_space="Shared")
            tabs[(l, g)] = (tin, tout, w)
    hfin_in = nc.dram_tensor("hfin_in", [D_H, SH], F32)
    hfin_out = nc.dram_tensor("hfin_out", [NCORES * D_H, SH], F32,
                              addr_space="Shared")
    cs_in = [nc.dram_tensor(f"cs_in_{i}", [1, NP], F32) for i in range(5)]
    scr_nrm = nc.dram_tensor("scr_nrm", [1, SH], F32)
    scr_inv = nc.dram_tensor("scr_inv", [1, SH], F32)
    scr_r = nc.dram_tensor("scr_r", [1, SH], F32)
    scr_c = nc.dram_tensor("scr_c", [1, NP], F32)
    cs_out = [nc.dram_tensor(f"cs_out_{i}", [1, NP], F32,
                             addr_space="Shared") for i in range(5)]

    RG = [list(range(NCORES))]

    with tile.TileContext(nc) as tc:
        with (
            tc.tile_pool(name="persist", bufs=1) as P,
            tc.tile_pool(name="mid", bufs=1) as MID,
            tc.tile_pool(name="psB", bufs=1, space="PSUM") as PSB,
            tc.tile_pool(name="psC", bufs=2, space="PSUM") as PSC,
        ):
            # iota / identity generated on device
            ii32 = P.tile([128, WIN], I32)
            nc.gpsimd.iota(ii32[:], pattern=[[1, WIN]], base=0,
                           channel_multiplier=0)
            iota = P.tile([128, WIN], F32)
            nc.vector.tensor_copy(iota[:], ii32[:])
            ci32 = P.tile([128, 128], I32)
            nc.gpsimd.iota(ci32[:], pattern=[[1, 128]], base=0,
                           channel_multiplier=-1)
            cif = P.tile([128, 128], F32)
            nc.vector.tensor_copy(cif[:], ci32[:])
            ident = P.tile([128, 128], F32)
            nc.vector.tensor_scalar(ident[:], cif[:], 0.0, None,
                                    mybir.AluOpType.is_equal)

            misc = P.tile([128, MISC_COLS], F32)
            nc.sync.dma_start(misc[:], misc_in[:])

            ebias = P.tile([128, 1], F32)
            nc.vector.memset(ebias[:], ALPHA * EPS)
            ones = P.tile([128, 1], F32)
            nc.vector.memset(ones[:], 1.0)
            ones1 = P.tile([1, 128], F32)
            nc.vector.memset(ones1[:], 1.0)
            hfinT = {}

            # ============ GNN phase (pool closes before sinkhorn) ========
            with (
                tc.tile_pool(name="gnn", bufs=1) as G,
                tc.tile_pool(name="work", bufs=1) as W,
                tc.tile_pool(name="vals", bufs=3) as V,
                tc.tile_pool(name="psA", bufs=1, space="PSUM") as PSA,
            ):
                # broadcast the 8-way-sharded weight pack (collectives
                # cannot read IO tensors; bounce through internal DRAM)
                nc.sync.dma_start(wpk_b.ap(), wpk_in.ap())
                nc.gpsimd.collective_compute(
                    "AllGather", mybir.AluOpType.bypass,
                    replica_groups=RG,
                    ins=[wpk_b.ap().opt()], outs=[wpk_out.ap().opt()])

                wt = {}
                for l in range(N_LAYERS):
                    f = FAN[l]
                    for nm in ("W1", "W2", "Wr"):
                        t = G.tile([128, f // 128, D_H], F32, tag=f"{nm}_{l}")
                        woff = WOFF[f"{nm}_{l}"]
                        for kt in range(f // 128):
                            nc.sync.dma_start(
                                t[:, kt, :],
                                wpk_out[woff + kt * 128:woff + (kt + 1) * 128,
                                        :])
                        wt[f"{nm}_{l}"] = t
                fw = G.tile([128, 7, D_H], F32)
                for kt in range(7):
                    woff = WOFF["final_w"]
                    nc.sync.dma_start(
                        fw[:, kt, :],
                        wpk_out[woff + kt * 128:woff + (kt + 1) * 128, :])

                rel8 = G.tile([128, rel_cols], U8)
                nc.sync.dma_start(rel8[:], relp[:])

                em = {}
                masks = {}
                ioff = 0
                roff = 0
                for g in ("s", "t"):
                    for d in range(2):
                        m = metas[g][d]
                        e = m["nblk"] * 128
                        it = G.tile([128, e // 16], I16, tag=f"idx_{g}{d}")
                        for grp in range(8):
                            nc.sync.dma_start(
                                it[grp * 16:(grp + 1) * 16, :],
                                idxp[:, ioff:ioff + e // 16])
                        ioff += e // 16
                        rl = G.tile([128, m["nblk"]], F32, tag=f"rel_{g}{d}")
                        nc.vector.tensor_copy(
                            rl[:], rel8[:, roff:roff + m["nblk"]])
                        roff += m["nblk"]
                        em[(g, d)] = (it, rl, DG_BASE[(g, d)], m)
                        mk = G.tile([128, m["nblk"], WIN], BF16,
                                    tag=f"mask_{g}{d}")
                        for b in range(m["nblk"]):
                            nc.vector.tensor_scalar(
                                mk[:, b, :], iota[:], rl[:, b:b + 1], None,
                                mybir.AluOpType.is_equal)
                        masks[(g, d)] = mk

                hT = {}
                for g, r0 in (("s", 0), ("t", SH)):
                    t = G.tile([128, 1, SH], F32, tag=f"hT0_{g}")
                    nc.sync.dma_start(
                        t[:, 0, :],
                        xin[r0:r0 + SH, :].rearrange("n f -> f n"))
                    hT[g] = t
                hist = {"s": [], "t": []}

                def write_table(l, g):
                    tin, tout, wdt = tabs[(l, g)]
                    f = FAN[l]
                    nm_t = W.tile([128, SH // 128, f], F32, tag="tab_nm")
                    for kt in range(f // 128):
                        for ntile in range(SH // 128):
                            pst = PSC.tile([128, 128], F32, tag="tr")
                            nc.tensor.transpose(
                                pst[:],
                                hT[g][:, kt, ntile * 128:(ntile + 1) * 128],
                                ident[:])
                            nc.scalar.copy(
                                nm_t[:, ntile, kt * 128:(kt + 1) * 128],
                                pst[:])
                    hi = W.tile([128, SH // 128, f], BF16, tag="tab_hi")
                    lo_f = W.tile([128, SH // 128, f], F32, tag="tab_lof")
                    lo = W.tile([128, SH // 128, f], BF16, tag="tab_lo")
                    nc.vector.tensor_copy(hi[:], nm_t[:])
                    nc.vector.tensor_tensor(lo_f[:], nm_t[:], hi[:],
                                            mybir.AluOpType.subtract)
                    nc.vector.tensor_copy(lo[:], lo_f[:])
                    for ntile in range(SH // 128):
                        nc.sync.dma_start(
                            tin[ntile * 128:(ntile + 1) * 128, 0:f],
                            hi[:, ntile, :])
                        nc.sync.dma_start(
                            tin[ntile * 128:(ntile + 1) * 128, f:2 * f],
                            lo[:, ntile, :])
                    nc.gpsimd.collective_compute(
                        "AllGather", mybir.AluOpType.bypass,
                        replica_groups=RG,
                        ins=[tin.ap().opt()], outs=[tout.ap().opt()])

                def aggregate(g, d, l):
                    tin, tout, wdt = tabs[(l, g)]
                    f = FAN[l]
                    it, rl, dgb, m = em[(g, d)]
                    mk = masks[(g, d)]
                    nblk = m["nblk"]
                    blk_win = m["blk_win"]
                    pst = [PSA.tile([128, f], F32, tag=f"agg{q}",
                                    name=f"aggps_{g}{d}{l}_{q}")
                           for q in range(4)]
                    started = [False] * NGRP
                    for c in range(nblk // CHUNK_BLK):
                        vhi = V.tile([128, CHUNK_BLK, f], BF16, tag="vhi")
                        vlo = V.tile([128, CHUNK_BLK, f], BF16, tag="vlo")
                        i0 = c * CHUNK_BLK * 128 // 16
                        i1 = (c + 1) * CHUNK_BLK * 128 // 16
                        for vi, (vt, foff) in enumerate(((vhi, 0),
                                                         (vlo, f))):
                            nc.gpsimd.dma_gather(
                                vt[:], tout[:, foff:foff + f], it[:, i0:i1],
                                CHUNK_BLK * 128, CHUNK_BLK * 128, f,
                                elem_step=wdt, single_packet=False,
                                queue_num=(2 * c + vi) % 4)
                        for bb in range(CHUNK_BLK):
                            b = c * CHUNK_BLK + bb
                            w = int(blk_win[b])
                            q, half = w // 2, w % 2
                            st = not started[w]
                            started[w] = True
                            last = (b == nblk - 1 or blk_win[b + 1] != w)
                            nc.tensor.matmul(
                                pst[q][half * 64:(half + 1) * 64, :],
                                mk[:, b, :], vhi[:, bb, :], start=st,
                                stop=False)
                            nc.tensor.matmul(
                                pst[q][half * 64:(half + 1) * 64, :],
                                mk[:, b, :], vlo[:, bb, :], start=False,
                                stop=last)
                    agg = W.tile([128, SH // 128, f], F32, tag=f"agg_nm{d}")
                    for q in range(SH // 128):
                        nc.vector.tensor_scalar_mul(
                            agg[:, q, :], pst[q][:],
                            misc[:, dgb + q:dgb + q + 1])
                    return agg

                def to_featmajor(agg, f, tag):
                    at = W.tile([128, f // 128, SH], F32, tag=tag)
                    for kt in range(f // 128):
                        for ntile in range(SH // 128):
                            pst = PSC.tile([128, 128], F32, tag="tr")
                            nc.tensor.transpose(
                                pst[:], agg[:, ntile, kt * 128:(kt + 1) * 128],
                                ident[:])
                            nc.scalar.copy(
                                at[:, kt, ntile * 128:(ntile + 1) * 128],
                                pst[:])
                    return at

                write_table(0, "s")
                write_table(0, "t")
                for l in range(N_LAYERS):
                    f = FAN[l]
                    for g in ("s", "t"):
                        hist[g].append(hT[g])
                        a1 = aggregate(g, 0, l)
                        a2 = aggregate(g, 1, l)
                        a1t = to_featmajor(a1, f, "a1t")
                        a2t = to_featmajor(a2, f, "a2t")
                        hn = G.tile([128, 2, SH], F32, tag=f"hT{l + 1}_{g}")
                        for mt in range(2):
                            pp = PSB.tile([128, SH], F32, tag="pre")
                            for kt in range(f // 128):
                                nc.tensor.matmul(
                                    pp[:],
                                    wt[f"Wr_{l}"][:, kt, mt * 128:(mt + 1) * 128],
                                    hT[g][:, kt, :], start=(kt == 0),
                                    stop=False)
                            for kt in range(f // 128):
                                nc.tensor.matmul(
                                    pp[:],
                                    wt[f"W1_{l}"][:, kt, mt * 128:(mt + 1) * 128],
                                    a1t[:, kt, :], start=False, stop=False)
                            for kt in range(f // 128):
                                nc.tensor.matmul(
                                    pp[:],
                                    wt[f"W2_{l}"][:, kt, mt * 128:(mt + 1) * 128],
                                    a2t[:, kt, :], start=False,
                                    stop=(kt == f // 128 - 1))
                            nc.scalar.activation(
                                hn[:, mt, :], pp[:],
                                mybir.ActivationFunctionType.Relu,
                                bias=misc[:, 2 * l + mt:2 * l + mt + 1],
                                scale=1.0)
                        hT[g] = hn
                        if l + 1 < N_LAYERS:
                            write_table(l + 1, g)

                # final linear + l2norm; t first so its AllGather overlaps
                # the s-side final compute
                for g in ("t", "s"):
                    hist[g].append(hT[g])
                    rhs = []
                    for t in hist[g]:
                        for kt in range(t[:].shape[1]):
                            rhs.append(t[:, kt, :])
                    hf = MID.tile([128, 2, SH], F32, tag=f"hfin_{g}")
                    for mt in range(2):
                        pp = PSB.tile([128, SH], F32, tag="pre")
                        for kt in range(7):
                            nc.tensor.matmul(
                                pp[:], fw[:, kt, mt * 128:(mt + 1) * 128],
                                rhs[kt], start=(kt == 0), stop=(kt == 6))
                        nc.scalar.copy(hf[:, mt, :], pp[:])
                        nc.vector.tensor_scalar_add(
                            hf[:, mt, :], hf[:, mt, :],
                            misc[:, 6 + mt:7 + mt])
                    sq = W.tile([128, 2, SH], F32, tag="sq")
                    nc.scalar.activation(sq[:, 0, :], hf[:, 0, :],
                                         mybir.ActivationFunctionType.Square)
                    nc.scalar.activation(sq[:, 1, :], hf[:, 1, :],
                                         mybir.ActivationFunctionType.Square)
                    nrm = PSA.tile([1, SH], F32, tag="nrm")
                    nc.tensor.matmul(nrm[:], ones[:], sq[:, 0, :], start=True,
                                     stop=False)
                    nc.tensor.matmul(nrm[:], ones[:], sq[:, 1, :], start=False,
                                     stop=True)
                    nrs = W.tile([1, SH], F32, tag="nrs")
                    nc.scalar.activation(nrs[:], nrm[:],
                                         mybir.ActivationFunctionType.Sqrt)
                    nr2 = W.tile([128, SH // 128], F32, tag="nr2")
                    nc.sync.dma_start(scr_nrm.ap(), nrs[:])
                    nc.sync.dma_start(
                        nr2[:], scr_nrm[0, :].rearrange("(c b) -> b c", b=128))
                    nc.vector.tensor_scalar_max(nr2[:], nr2[:], 1e-12)
                    inv = W.tile([128, SH // 128], F32, tag="inv")
                    nc.vector.reciprocal(inv[:], nr2[:])
                    if g == "s":
                        nc.vector.tensor_tensor(inv[:], inv[:], misc[:, 8:12],
                                                mybir.AluOpType.mult)
                    invr = W.tile([1, SH], F32, tag="invr")
                    nc.sync.dma_start(
                        scr_inv[0, :].rearrange("(c b) -> b c", b=128), inv[:])
                    nc.sync.dma_start(invr[:], scr_inv.ap())
                    invb = W.tile([128, SH], F32, tag="invb")
                    bcp = PSA.tile([128, SH], F32, tag="nrm", name="bcp")
                    nc.tensor.matmul(bcp[:], ones1[:], invr[:], start=True,
                                     stop=True)
                    nc.vector.tensor_copy(invb[:], bcp[:])
                    for mt in range(2):
                        nc.vector.tensor_tensor(hf[:, mt, :], hf[:, mt, :],
                                                invb[:], mybir.AluOpType.mult)
                    hfinT[g] = hf
                    if g == "t":
                        for mt in range(2):
                            nc.sync.dma_start(
                                hfin_in[mt * 128:(mt + 1) * 128, :],
                                hf[:, mt, :])
                        nc.gpsimd.collective_compute(
                            "AllGather", mybir.AluOpType.bypass,
                            replica_groups=RG,
                            ins=[hfin_in.ap().opt()],
                            outs=[hfin_out.ap().opt()])

            # ============ match + sinkhorn phase ============

            with (
                tc.tile_pool(name="sink", bufs=1) as S,
                tc.tile_pool(name="work2", bufs=1) as W2,
                tc.tile_pool(name="psS", bufs=1, space="PSUM") as PSS,
            ):
                m2_pool = tc.tile_pool(name="m2", bufs=1)
                M2 = m2_pool.__enter__()
                htn = M2.tile([128, 2, NP], F32)
                for r in range(NCORES):
                    nc.sync.dma_start(
                        htn[:, :, r * SH:(r + 1) * SH],
                        hfin_out[r * D_H:(r + 1) * D_H, :].rearrange(
                            "(h p) c -> p h c", p=128))
                S0 = [S.tile([128, NP], F32, tag=f"S0_{q}", name=f"S0_{q}")
                      for q in range(4)]
                T0 = [S.tile([128, SH], F32, tag=f"T0_{q}", name=f"T0_{q}")
                      for q in range(32)]
                for q in range(4):
                    for nchk in range(NP // 512):
                        pp = PSB.tile([128, SH], F32, tag="pre")
                        for kt in range(2):
                            nc.tensor.matmul(
                                pp[:], hfinT["s"][:, kt, q * 128:(q + 1) * 128],
                                htn[:, kt, nchk * 512:(nchk + 1) * 512],
                                start=(kt == 0), stop=(kt == 1))
                        nc.scalar.activation(
                            S0[q][:, nchk * 512:(nchk + 1) * 512], pp[:],
                            mybir.ActivationFunctionType.Exp,
                            bias=ebias[:, 0:1], scale=ALPHA)
                for q in range(4):
                    for jt in range(32):
                        pst = PSC.tile([128, 128], F32, tag="tr")
                        nc.tensor.transpose(
                            pst[:], S0[q][:, jt * 128:(jt + 1) * 128], ident[:])
                        if jt % 2 == 0:
                            nc.scalar.copy(T0[jt][:, q * 128:(q + 1) * 128],
                                           pst[:])
                        else:
                            nc.vector.tensor_copy(
                                T0[jt][:, q * 128:(q + 1) * 128], pst[:])

                m2_pool.__exit__(None, None, None)
                rt = S.tile([128, 4], F32)
                ct = S.tile([128, 32], F32)
                nc.vector.memset(rt[:], 1.0)
                csum_i = 0
                for it_i in range(SINK_ITERS):
                    if it_i % 2 == 0:
                        part = W2.tile([1, NP], F32, tag="part")
                        for nchk in range(NP // 512):
                            pp = PSS.tile([1, 512], F32, tag="cs")
                            for q in range(4):
                                nc.tensor.matmul(
                                    pp[:], rt[:, q:q + 1],
                                    S0[q][:, nchk * 512:(nchk + 1) * 512],
                                    start=(q == 0), stop=(q == 3))
                            nc.scalar.copy(
                                part[:, nchk * 512:(nchk + 1) * 512], pp[:])
                        nc.sync.dma_start(cs_in[csum_i][:], part[:])
                        nc.gpsimd.collective_compute(
                            "AllReduce", mybir.AluOpType.add,
                            replica_groups=RG,
                            ins=[cs_in[csum_i].ap().opt()],
                            outs=[cs_out[csum_i].ap().opt()])
                        ssum = W2.tile([128, 32], F32, tag="ssum")
                        nc.sync.dma_start(
                            ssum[:],
                            cs_out[csum_i][0, :].rearrange("(f p) -> p f",
                                                           p=128))
                        nc.vector.reciprocal(ct[:], ssum[:])
                        csum_i += 1
                    else:
                        pp = PSS.tile([1, SH], F32, tag="rs")
                        for jt in range(32):
                            nc.tensor.matmul(pp[:], ct[:, jt:jt + 1], T0[jt][:],
                                             start=(jt == 0), stop=(jt == 31))
                        rr = W2.tile([1, SH], F32, tag="rr")
                        nc.scalar.copy(rr[:], pp[:])
                        r2 = W2.tile([128, 4], F32, tag="r2")
                        nc.sync.dma_start(scr_r.ap(), rr[:])
                        nc.sync.dma_start(
                            r2[:], scr_r[0, :].rearrange("(c b) -> b c", b=128))
                        nc.vector.reciprocal(rt[:], r2[:])

                fin_cm = tc.tile_pool(name="fin", bufs=1)
                FIN = fin_cm.__enter__()
                crow = FIN.tile([1, NP], F32, tag="crow")
                nc.sync.dma_start(
                    scr_c[0, :].rearrange("(c b) -> b c", b=128), ct[:])
                nc.sync.dma_start(crow[:], scr_c.ap())
                cb = FIN.tile([128, NP], F32, tag="cb")
                for ch in range(NP // 512):
                    cbp = PSS.tile([128, 512], F32, tag="cb", name="cbp")
                    nc.tensor.matmul(cbp[:], ones1[:],
                                     crow[:, ch * 512:(ch + 1) * 512],
                                     start=True, stop=True)
                    nc.vector.tensor_copy(cb[:, ch * 512:(ch + 1) * 512],
                                          cbp[:])
                for q in range(4):
                    outt = FIN.tile([128, NP], BF16, tag="outt")
                    nc.vector.scalar_tensor_tensor(
                        outt[:], S0[q][:], rt[:, q:q + 1], cb[:],
                        mybir.AluOpType.mult, mybir.AluOpType.mult)
                    nc.sync.dma_start(out_rows[q * 128:(q + 1) * 128, :],
                                      outt[:])
                fin_cm.__exit__(None, None, None)

    nc.compile()
    return nc


def kernel(**inputs):
    x_s = np.asarray(inputs["x_s"], np.float32)
    x_t = np.asarray(inputs["x_t"], np.float32)
    meta_s = _prep_edges(np.asarray(inputs["edges"]))
    meta_t = _prep_edges(np.asarray(inputs["edget"]))
    nc = build_program(meta_s, meta_t)

    xs_pad = np.zeros((NP, D_IN), np.float32)
    xs_pad[:NS] = x_s
    xt_pad = x_t

    # canonical packed weights [WPAD, 256]
    wpk = np.zeros((WPAD, D_H), np.float32)
    for l in range(N_LAYERS):
        for nm in ("W1", "W2", "Wr"):
            w = np.asarray(inputs[f"{nm}_{l}"], np.float32)
            wpk[WOFF[f"{nm}_{l}"]:WOFF[f"{nm}_{l}"] + w.shape[0]] = w
    fwv = np.asarray(inputs["final_w"], np.float32)
    wpk[WOFF["final_w"]:WOFF["final_w"] + fwv.shape[0]] = fwv

    in_maps = []
    for k in range(NCORES):
        misc = np.zeros((128, MISC_COLS), np.float32)
        for l in range(N_LAYERS):
            misc[:, 2 * l:2 * l + 2] = np.asarray(
                inputs[f"br_{l}"], np.float32).reshape(2, 128).T
        misc[:, 6:8] = np.asarray(inputs["final_b"],
                                  np.float32).reshape(2, 128).T
        vld = np.zeros(SH, np.float32)
        n_real = max(0, min(SH, NS - k * SH))
        vld[:n_real] = 1.0
        misc[:, 8:12] = vld.reshape(SH // 128, 128).T
        idx_parts, rel_parts = [], []
        for g, meta in (("s", meta_s), ("t", meta_t)):
            for d in range(2):
                md = meta[d]
                idx_parts.append(_wrap_idx(md["idx"][k]))
                rel_parts.append(_rel_tile(md["rel"][k]))
                dgk = md["inv_deg"][k * SH:(k + 1) * SH]
                misc[:, DG_BASE[(g, d)]:DG_BASE[(g, d)] + 4] = \
                    dgk.reshape(SH // 128, 128).T
        m = dict(
            xin=np.ascontiguousarray(np.concatenate(
                [xs_pad[k * SH:(k + 1) * SH], xt_pad[k * SH:(k + 1) * SH]],
                axis=0)),
            wpk_in=np.ascontiguousarray(wpk[k * WSH:(k + 1) * WSH]),
            misc_in=misc,
            idxp=np.ascontiguousarray(np.concatenate(idx_parts, axis=1)),
            relp=np.ascontiguousarray(np.concatenate(rel_parts, axis=1)),
        )
        in_maps.append(m)

    res = run_bass_kernel_spmd(nc, in_maps, list(range(NCORES)))
    rows = np.concatenate(
        [np.asarray(res.results[k]["out_rows"]).astype(np.float32)
         for k in range(NCORES)], axis=0)
    kernel._last = (nc, in_maps)
    return rows[:NS].astype(np.float32)
